# revision 10
# baseline (speedup 1.0000x reference)
"""Trainium2 Bass kernel for sparse (shared-prefix) GQA decode attention.

Full-input contract: kernel(**inputs) takes the unsharded tensors from
setup_inputs() and returns the full [16, 1, 4096] float32 output.

Sharding: tensor-parallel over heads across 8 NeuronCores. Core m owns
query heads 4m..4m+3 and kv head m (GQA group m): wq columns
[512m, 512m+512), wk/wv columns [128m, 128m+128), wo rows [512m, 512m+512),
and head m of the kv caches. Each core computes a partial y_m = attn_m @
wo_m; the host sums the 8 partials (the "all-reduce").

Design notes:
  * kv cache and wk/wv stream in fp8 e3m4 (4-bit mantissa); wq/wo stay
    bf16. Mixed-dtype matmuls (fp8 stationary x bf16 moving) are legal on
    TRN2, so q and the probabilities stay bf16. ~15.6MB/core HBM traffic.
  * RoPE is folded into wq/wk on the host (seqlen=1: one fixed rotation,
    a host-side weight reparameterization): no rope, no transposes on
    device. Projections run weight-stationary and produce qT/xkT/xvT
    directly in [d, batch] orientation.
  * PV runs v-stationary (lhsT = v chunk [j,128d], moving = probs
    [j,4h]); attention lands as attnT [128d, (b,h)] with no transposes.
  * kT and v DMA pieces are interleaved so QK and PV rounds alternate
    per 4-batch group and the PE trails the HBM stream by ~one piece.
  * Output projection packs the PE array 4x with column tiling
    (tile_position=(0,32j)): batch M=16 only fills 1/8 of the array, so
    four n-blocks run concurrently in separate column groups.
  * Input DMAs are split across both HWDGE rings (SP + ACT) in
    consumption order; everything is SBUF-resident.
  * PSUM rule learned on HW: start=True resets has_written for the WHOLE
    bank, so concurrently-accumulating groups live in separate banks.

Problem constants (hardcoded per the harness contract): bsz=16, seqlen=1,
dim=4096, n_heads=32, n_kv=8, hd=128, start_pos=2048,
shared_prefix_length=512 -> rsp=1536, L=2049.
"""

import math
import os
import sys
import types

import numpy as np

# ----------------------------------------------------------------------------
# environment patches (self-contained; no /root/problem reads)
# ----------------------------------------------------------------------------


def _patch_tile_drain():
    """The stock TileContext._drain_and_barrier puts one sem-wait per live
    semaphore on a single Drain instruction; the walrus build in this image
    only accepts a single sync wait per instruction ("Too many sync wait
    commands"). Re-emit the waits as individual EventSemaphore instructions
    on the same sequencer instead."""
    import concourse.tile as tile
    from concourse.vector_clock import ScopedClock

    if getattr(tile.TileContext, "_drain_patched", False):
        return

    def _drain_and_barrier(self, tick_clock, wait_clock):
        nc = self.nc
        drain_inst = nc.sync.drain()
        wait_clock.add_sem_waits(
            drain_inst.ins, ScopedClock({None: tick_clock.global_clock})
        )
        waits = list(drain_inst.ins.sync_info.on_wait)
        if len(waits) > 1:
            by_name = {h.name: h for h in self.sems.allocated().values()}
            try:
                drain_inst.ins.sync_info = None
            except Exception:
                pass
            for w in waits:
                h = by_name.get(w.ant_name)
                assert h is not None, f"no handle for sem {w.ant_name}"
                nc.sync.wait_ge(h, w.wait_value)

        assert self.sems is not None
        popped = nc._tile_sem_poison_stack.pop()
        assert popped is self._sem_poison
        nums = [h.num for h in self.sems.allocated().values()]
        nc._state.prepend_free_semaphores(nums)
        for ps in nc._tile_sem_poison_stack:
            ps.update(nums)

    tile.TileContext._drain_and_barrier = _drain_and_barrier
    tile.TileContext._drain_patched = True


def _install_ntff_hook():
    """Optional: register the axon NTFF profile hook (missing from the
    trimmed antenv package) so trace=True works for profiling, and stub the
    S3 artifact upload (zero-egress container)."""
    try:
        if "antenv.axon_hooks" not in sys.modules:
            mod = types.ModuleType("antenv.axon_hooks")
            mod._hook = None
            mod.set_axon_ntff_profile_hook = lambda h: setattr(mod, "_hook", h)
            mod.get_axon_ntff_profile_hook = lambda: mod._hook
            sys.modules["antenv.axon_hooks"] = mod
            import antenv

            antenv.axon_hooks = mod
            from trn_agent_boot.trn_boot import _ntff_profile_via_ctypes

            mod.set_axon_ntff_profile_hook(
                _ntff_profile_via_ctypes("/opt/axon/libaxon_pjrt.so")
            )
        import concourse.bass_utils as bu

        bu.upload_artifacts = lambda tmpdir: tmpdir
    except Exception:
        pass


def _legalize_multiwait(nc, max_waits=1):
    """This walrus build accepts at most one sync wait per instruction.
    Hoist excess waits into standalone single-wait EventSemaphore
    instructions inserted immediately before, on the same engine."""
    import bass_rust

    uid = 0
    for f in nc.m.functions:
        for bb in f.blocks:
            insts = list(bb.instructions)
            out = []
            changed = False
            for ins in insts:
                si = ins.sync_info
                if si is not None:
                    waits = list(si.on_wait)
                    if len(waits) > max_waits:
                        for w in waits[:-max_waits]:
                            ev = bass_rust.InstEventSemaphore(
                                name=f"{ins.name}_xw{uid}"
                            )
                            uid += 1
                            ev.engine = ins.engine
                            ev.sync_info = bass_rust.SyncInfo(
                                on_wait=[w], on_update=[]
                            )
                            out.append(ev)
                        ins.sync_info = bass_rust.SyncInfo(
                            on_wait=waits[-max_waits:],
                            on_update=list(si.on_update),
                        )
                        changed = True
                out.append(ins)
            if changed:
                bb.instructions = out


# ----------------------------------------------------------------------------
# constants
# ----------------------------------------------------------------------------

N_CORES = 8
B = 16            # batch
DIM = 4096
N_HEADS = 32
N_KV = 8
HD = 128
NH = N_HEADS // N_CORES      # 4 local q heads
R = B * NH                   # 64 cols, r = 4*b + h
SOFTMAX_SCALE = 1.0 / math.sqrt(HD)
WS = 32.0                    # host pre-scale on wk/wv so e3m4 normals cover them

# stream dtypes (env-overridable for A/B tests)
CACHE_DT = os.environ.get("KERNEL_CACHE_DT", "float8e3")
WKV_DT = os.environ.get("KERNEL_WKV_DT", "float8e3")
WQ_DT = os.environ.get("KERNEL_WQ_DT", "bfloat16")
WO_DT = os.environ.get("KERNEL_WO_DT", "bfloat16")

WARMUP_MMS = int(os.environ.get("KERNEL_WARMUP_MMS", "36"))
WO_TILES = int(os.environ.get("KERNEL_WO_TILES", "4"))  # col-tiling width


# ----------------------------------------------------------------------------
# device kernel
# ----------------------------------------------------------------------------


def _build_nc(spl, rsp):
    import concourse.bass as bass
    import concourse.tile as tile
    from concourse import mybir
    from concourse.mybir import ActivationFunctionType as AF

    BF = mybir.dt.bfloat16
    F16 = mybir.dt.float16
    f32 = mybir.dt.float32
    CDT = getattr(mybir.dt, CACHE_DT)
    KVDT = getattr(mybir.dt, WKV_DT)
    QDT = getattr(mybir.dt, WQ_DT)
    ODT = getattr(mybir.dt, WO_DT)

    assert spl % 128 == 0 and rsp % (128 * 4) == 0
    SH_CH = spl // 128           # shared j-chunks (4)
    BCH = rsp // 128             # per-batch cache j-chunks (12)
    NCH = SH_CH + BCH + 1        # total chunks incl. new-token chunk (17)
    NPC = 4                      # batches per kT/v DMA piece
    NPIECE = B // NPC            # 4 pieces

    nc = bass.Bass(
        "TRN2", target_bir_lowering=False, debug=False, num_devices=N_CORES
    )

    cpack_d = nc.dram_tensor("cpack", [128, 32 * B + 2 * spl], BF,
                             kind="ExternalInput").ap()
    wq_d = nc.dram_tensor("wq", [4, 128, 8 * NH * HD], QDT,
                          kind="ExternalInput").ap()
    wkv_d = nc.dram_tensor("wkv", [128, 32 * 2 * HD], KVDT,
                           kind="ExternalInput").ap()
    kT_d = nc.dram_tensor("kT", [NPIECE, 128, NPC * rsp], CDT,
                          kind="ExternalInput").ap()
    v_d = nc.dram_tensor("v", [NPIECE, 128, NPC * rsp], CDT,
                         kind="ExternalInput").ap()
    wo_d = nc.dram_tensor("wo", [NH, 128, DIM], ODT,
                          kind="ExternalInput").ap()
    y_d = nc.dram_tensor("y", [B, DIM], f32, kind="ExternalOutput").ap()

    with tile.TileContext(nc) as tc:
        with tc.tile_pool(name="const", bufs=1) as const:
            # ---------------- resident SBUF tiles ----------------
            ones_sb = const.tile([128, 1], BF, tag="ones")
            ones1p = const.tile([1, 128], BF, tag="ones1p")
            g64 = const.tile([128, R], BF, tag="g64")
            scr1 = const.tile([1, 4], BF, tag="scr1")
            cpack_sb = const.tile([128, 32 * B + 2 * spl], BF, tag="cpack")
            wq_sb = const.tile([128, 32 * NH * HD], QDT, tag="wq")
            wkv_sb = const.tile([128, 32 * 2 * HD], KVDT, tag="wkv")
            kT_sb = const.tile([128, B * rsp], CDT, tag="kT")
            v_sb = const.tile([128, B * rsp], CDT, tag="v")
            wo_sb = const.tile([128, NH * DIM], ODT, tag="wo")
            qT_sb = const.tile([128, R], BF, tag="qT")
            xkT_sb = const.tile([128, B], BF, tag="xkT")
            xvT_sb = const.tile([128, B], BF, tag="xvT")
            xk_bc = const.tile([128, R], BF, tag="xk_bc")
            xv_bc = const.tile([128, R], BF, tag="xv_bc")
            prod_sb = const.tile([128, R], F16, tag="prod")
            pT = const.tile([128, NCH, R], BF, tag="pT")
            sum1 = const.tile([1, R], f32, tag="sum1")
            rinv1 = const.tile([1, R], BF, tag="rinv1")
            rbc_sb = const.tile([128, 2, R], BF, tag="rbc")  # [pnew_bc|rinv_bc]
            attnT = const.tile([128, R], BF, tag="attnT")    # cols 4b+h
            attnT2 = const.tile([128, R], BF, tag="attnT2")  # cols 16h+b
            y_sb = const.tile([128, 2 * 512], f32, tag="y")  # band j=n%4 rows

            xT = cpack_sb[:, : 32 * B]
            shkT = cpack_sb[:, 32 * B : 32 * B + spl]
            shv = cpack_sb[:, 32 * B + spl :]

            # ---------------- constants ----------------
            nc.vector.memset(ones_sb, 1.0)
            nc.vector.memset(ones1p, 1.0)
            nc.vector.memset(g64, 0.0)
            nc.vector.memset(pT[:, NCH - 1, :], 0.0)

            # ---------------- input DMA stream ----------------
            # consumption order: x/shared, wq, wkv, (kT, v interleaved), wo
            # alternated across the two HWDGE rings (SP and ACT).
            WQP = 8 * NH * HD
            KVP = NPC * rsp
            nc.scalar.dma_start(out=cpack_sb, in_=cpack_d)
            for g in range(4):
                eng = nc.sync if g % 2 == 0 else nc.scalar
                eng.dma_start(out=wq_sb[:, WQP * g : WQP * (g + 1)],
                              in_=wq_d[g])
            nc.sync.dma_start(out=wkv_sb, in_=wkv_d)
            for g in range(NPIECE):
                nc.sync.dma_start(out=kT_sb[:, KVP * g : KVP * (g + 1)],
                                  in_=kT_d[g])
                nc.scalar.dma_start(out=v_sb[:, KVP * g : KVP * (g + 1)],
                                    in_=v_d[g])
            for h in range(NH):
                eng = nc.sync if h % 2 == 0 else nc.scalar
                eng.dma_start(out=wo_sb[:, DIM * h : DIM * (h + 1)],
                              in_=wo_d[h])

            # preload the ACT Exp table during the DMA dead time
            nc.scalar.activation(out=scr1, in_=g64[0:1, 0:4], func=AF.Exp)

            # ---------------- PE warmup (HAM clock gate) ----------------
            with tc.tile_pool(name="pwarm", bufs=1, space="PSUM") as pw:
                wps = pw.tile([1, R], f32, tag="wps")
                for i in range(WARMUP_MMS):
                    nc.tensor.matmul(wps, ones_sb, g64, start=True, stop=True)

            # ---------------- long-lived PSUM pools ----------------
            with tc.tile_pool(name="ppv", bufs=1, space="PSUM") as ppvp, \
                 tc.tile_pool(name="psum1", bufs=1, space="PSUM") as psump, \
                 tc.tile_pool(name="pnew", bufs=1, space="PSUM") as pnewp:
                ps_pv = ppvp.tile([128, R], f32, tag="pv")
                ps_sum = psump.tile([1, R], f32, tag="sum")
                ps_snew = pnewp.tile([1, R], f32, tag="snew")
                ps_bc = pnewp.tile([128, 2, R], f32, tag="bc")

                # ---------------- q projection (weight-stationary) --------
                # NB: start=True resets has_written for the WHOLE psum bank,
                # so every concurrently-accumulating group needs its own
                # bank (PSUM pool slots are bank-granular).
                with tc.tile_pool(name="psq", bufs=1, space="PSUM") as psqp:
                    psq = [psqp.tile([128, B], f32, tag=f"psq{h}",
                                     name=f"psq{h}") for h in range(NH)]
                    for kc in range(32):
                        rx = xT[:, B * kc : B * (kc + 1)]
                        for h in range(NH):
                            nc.tensor.matmul(
                                psq[h],
                                wq_sb[:, (kc * NH + h) * HD : (kc * NH + h + 1) * HD],
                                rx,
                                start=(kc == 0),
                                stop=(kc == 31),
                            )
                    qTv = qT_sb.rearrange("p (b h) -> p b h", h=NH)
                    for h in range(NH):
                        nc.vector.tensor_copy(qTv[:, :, h], psq[h])

                # ---------------- shared-prefix scores + PV ----------------
                with tc.tile_pool(name="psh", bufs=1, space="PSUM") as pshp:
                    ps_sh = pshp.tile([128, SH_CH, R], f32, tag="sh")
                    for c in range(SH_CH):
                        nc.tensor.matmul(
                            ps_sh[:, c, :],
                            shkT[:, 128 * c : 128 * (c + 1)],
                            qT_sb,
                            start=True, stop=True,
                        )
                    nc.scalar.activation(
                        out=pT[:, 0:SH_CH, :], in_=ps_sh,
                        func=AF.Exp, scale=SOFTMAX_SCALE,
                    )
                # shared PV opens the big accumulation into ps_pv
                for c in range(SH_CH):
                    nc.tensor.matmul(
                        ps_pv,
                        shv[:, 128 * c : 128 * (c + 1)],
                        pT[:, c, :],
                        start=(c == 0), stop=False,
                        skip_group_check=True,
                    )
                # shared part of the softmax denominators
                for c in range(SH_CH):
                    nc.tensor.matmul(
                        ps_sum, ones_sb, pT[:, c, :],
                        start=(c == 0), stop=False,
                        skip_group_check=True,
                    )

                # ---------------- k/v projection ----------------
                with tc.tile_pool(name="pskv", bufs=1, space="PSUM") as pskvp:
                    pskv = [pskvp.tile([128, B], f32, tag=f"pskv{u}",
                                       name=f"pskv{u}") for u in range(2)]
                    for kc in range(32):
                        rx = xT[:, B * kc : B * (kc + 1)]
                        for u in range(2):
                            nc.tensor.matmul(
                                pskv[u],
                                wkv_sb[:, (kc * 2 + u) * HD : (kc * 2 + u + 1) * HD],
                                rx,
                                start=(kc == 0),
                                stop=(kc == 31),
                            )
                    nc.scalar.activation(out=xkT_sb, in_=pskv[0],
                                         func=AF.Copy, scale=1.0 / WS)
                    nc.scalar.activation(out=xvT_sb, in_=pskv[1],
                                         func=AF.Copy, scale=1.0 / WS)

                # new-token k/v broadcast + q.k product (DVE)
                xkv_ = xk_bc.rearrange("p (b h) -> p b h", h=NH)
                xvv_ = xv_bc.rearrange("p (b h) -> p b h", h=NH)
                for h in range(NH):
                    nc.vector.tensor_copy(xkv_[:, :, h], xkT_sb)
                    nc.vector.tensor_copy(xvv_[:, :, h], xvT_sb)
                nc.vector.tensor_mul(prod_sb, qT_sb, xk_bc)

                # ---------------- interleaved QK / PV rounds ----------------
                pTc = pT[:, SH_CH : SH_CH + BCH, :].rearrange(
                    "p c (g r2) -> p c g r2", r2=2 * NH
                )

                def pv_piece(g):
                    for j in range(NPC):
                        b = NPC * g + j
                        for c in range(BCH):
                            nc.tensor.matmul(
                                ps_pv[:, NH * b : NH * (b + 1)],
                                v_sb[:, rsp * b + 128 * c : rsp * b + 128 * (c + 1)],
                                pT[:, SH_CH + c, NH * b : NH * (b + 1)],
                                start=False, stop=(c == BCH - 1),
                                skip_group_check=True,
                            )

                with tc.tile_pool(name="pqk", bufs=3, space="PSUM") as pqkp:
                    for rnd in range(NPIECE):
                        for g2 in range(2):     # 2 batches per psum tile
                            grp = 2 * rnd + g2
                            qk = pqkp.tile([128, BCH, 2 * NH], f32, tag="qk",
                                           name=f"qk{grp}")
                            for b2 in range(2):
                                b = 2 * grp + b2
                                rq = qT_sb[:, NH * b : NH * (b + 1)]
                                for c in range(BCH):
                                    nc.tensor.matmul(
                                        qk[:, c, NH * b2 : NH * (b2 + 1)],
                                        kT_sb[:, rsp * b + 128 * c : rsp * b + 128 * (c + 1)],
                                        rq,
                                        start=True, stop=True,
                                    )
                            nc.scalar.activation(
                                out=pTc[:, :, grp, :], in_=qk,
                                func=AF.Exp, scale=SOFTMAX_SCALE,
                            )
                        if rnd == 0:
                            # new-token score + prob (fits the round-0 gap)
                            nc.tensor.matmul(ps_snew, ones_sb, prod_sb,
                                             start=True, stop=True)
                            nc.scalar.activation(
                                out=pT[0:1, NCH - 1, :], in_=ps_snew,
                                func=AF.Exp, scale=SOFTMAX_SCALE,
                            )
                        if rnd == 1:
                            nc.tensor.matmul(ps_bc[:, 0, :], ones1p,
                                             pT[0:1, NCH - 1, :],
                                             start=True, stop=True)
                        if rnd == NPIECE - 1:
                            # denominators: cache chunks + new-token chunk
                            for c in range(BCH):
                                nc.tensor.matmul(
                                    ps_sum, ones_sb, pT[:, SH_CH + c, :],
                                    start=False, stop=False,
                                    skip_group_check=True,
                                )
                            nc.tensor.matmul(
                                ps_sum, ones_sb, pT[:, NCH - 1, :],
                                start=False, stop=True, skip_group_check=True,
                            )
                            nc.vector.tensor_copy(sum1, ps_sum)
                            with nc.allow_low_precision(reason="1/rowsum"):
                                nc.vector.reciprocal(rinv1, sum1)
                        pv_piece(rnd)
                        # per-piece evacuation of the attention output
                        S = slice(NH * NPC * rnd, NH * NPC * (rnd + 1))
                        nc.scalar.activation(out=attnT[:, S],
                                             in_=ps_pv[:, S], func=AF.Copy)

                # 1/rowsum broadcast, new-token add, normalize, reorder
                nc.tensor.matmul(ps_bc[:, 1, :], ones1p, rinv1,
                                 start=True, stop=True)
                nc.vector.tensor_copy(rbc_sb, ps_bc)
                nc.vector.tensor_mul(xv_bc, xv_bc, rbc_sb[:, 0, :])
                nc.vector.tensor_add(attnT, attnT, xv_bc)
                a2v = attnT2.rearrange("p (h b) -> p b h", b=B)
                nc.vector.tensor_mul(
                    a2v,
                    attnT.rearrange("p (b h) -> p b h", h=NH),
                    rbc_sb[:, 1, :].rearrange("p (b h) -> p b h", h=NH),
                )

            # ---------------- output projection (col-tiled) ----------------
            # M=16 fills 1/8 of the PE array; run WO_TILES n-blocks
            # concurrently in separate 32-column groups of the array.
            NQ = 8 // WO_TILES          # quads per h-pass
            with tc.tile_pool(name="py", bufs=1, space="PSUM") as pyp:
                ys = [pyp.tile([128, 512], f32, tag=f"y{q}", name=f"ys{q}")
                      for q in range(NQ)]
                for h in range(NH):
                    lq = attnT2[:, B * h : B * (h + 1)]
                    for q in range(NQ):
                        for j in range(WO_TILES):
                            n = WO_TILES * q + j
                            nc.tensor.matmul(
                                ys[q][32 * j : 32 * j + B, :],
                                lq,
                                wo_sb[:, DIM * h + 512 * n : DIM * h + 512 * (n + 1)],
                                start=(h == 0),
                                stop=(h == NH - 1),
                                tile_position=(0, 32 * j),
                            )
                # evacuate bands and stream the output out piecewise
                for q in range(NQ):
                    for j in range(WO_TILES):
                        n = WO_TILES * q + j
                        band = slice(32 * j, 32 * j + B)
                        eng = nc.vector if j % 2 == 0 else nc.scalar
                        if eng is nc.vector:
                            nc.vector.tensor_copy(
                                y_sb[band, 512 * q : 512 * (q + 1)],
                                ys[q][band, :],
                            )
                        else:
                            nc.scalar.activation(
                                out=y_sb[band, 512 * q : 512 * (q + 1)],
                                in_=ys[q][band, :], func=AF.Copy,
                            )
                        deng = nc.sync if n % 2 == 0 else nc.scalar
                        deng.dma_start(
                            out=y_d[:, 512 * n : 512 * (n + 1)],
                            in_=y_sb[band, 512 * q : 512 * (q + 1)],
                        )

            if os.environ.get("KERNEL_DEBUG") == "1":
                def dbg(name, ap):
                    d = nc.dram_tensor(
                        f"dbg_{name}", list(ap.shape), ap.dtype,
                        kind="ExternalOutput",
                    ).ap()
                    nc.sync.dma_start(out=d, in_=ap)
                dbg("qT", qT_sb)
                dbg("xkT", xkT_sb)
                dbg("xvT", xvT_sb)
                dbg("pT", pT)
                dbg("sum1", sum1)
                dbg("rbc", rbc_sb)
                dbg("attnT", attnT)
                dbg("attnT2", attnT2)
                dbg("prod", prod_sb)

    if os.environ.get("KERNEL_SKIP_LEGALIZE") != "1":
        _legalize_multiwait(nc)
    return nc


# ----------------------------------------------------------------------------
# host-side sharding / layout prep
# ----------------------------------------------------------------------------


def _np_dt(name):
    import ml_dtypes

    return {
        "bfloat16": ml_dtypes.bfloat16,
        "float8e3": ml_dtypes.float8_e3m4,
        "float16": np.float16,
        "float32": np.float32,
    }[name]


def _prep_inputs(inputs, spl, rsp):
    x = np.asarray(inputs["x"], np.float32)            # [16, 1, 4096]
    wq = np.asarray(inputs["wq"], np.float32)
    wk = np.asarray(inputs["wk"], np.float32)
    wv = np.asarray(inputs["wv"], np.float32)
    wo = np.asarray(inputs["wo"], np.float32)
    ck = np.asarray(inputs["cache_k"], np.float32)     # [16, 4096, 8, 128]
    cv = np.asarray(inputs["cache_v"], np.float32)
    shk = np.asarray(inputs["shared_cache_k"], np.float32)  # [1, 512, 8, 128]
    shv = np.asarray(inputs["shared_cache_v"], np.float32)
    cos = np.asarray(inputs["freqs_cos"], np.float32)[0]    # [64]
    sin = np.asarray(inputs["freqs_sin"], np.float32)[0]

    bdt = _np_dt("bfloat16")
    cdt = _np_dt(CACHE_DT)
    kvdt = _np_dt(WKV_DT)
    qdt = _np_dt(WQ_DT)
    odt = _np_dt(WO_DT)

    def fold_rope(w):
        # seqlen=1 decode: rope is one fixed pairwise rotation; fold it
        # into the projection columns (a host-side reparameterization)
        W = w.reshape(w.shape[0], -1, 64, 2)
        we, wo_ = W[..., 0], W[..., 1]
        return np.stack(
            [we * cos - wo_ * sin, we * sin + wo_ * cos], -1
        ).reshape(w.shape)

    wq_r = fold_rope(wq)
    wk_r = fold_rope(wk) * WS
    wv_s = wv * WS

    xm = x[:, 0, :]                                    # [16, 4096]
    xT_p = np.ascontiguousarray(
        xm.T.reshape(32, 128, B).transpose(1, 0, 2)
    ).reshape(128, 32 * B)

    BCH = rsp // 128
    in_maps = []
    for m in range(N_CORES):
        # wq': pieces of 8 k-chunks; col ((kc%8)*4+h)*128+d
        wqm = wq_r[:, 512 * m : 512 * (m + 1)]         # [4096, 512]
        wq_p = np.ascontiguousarray(
            wqm.reshape(4, 8, 128, NH * HD).transpose(0, 2, 1, 3)
        ).reshape(4, 128, 8 * NH * HD).astype(qdt)

        # wkv': col (kc*2+u)*128+d
        wkvm = np.concatenate(
            [wk_r[:, 128 * m : 128 * (m + 1)], wv_s[:, 128 * m : 128 * (m + 1)]],
            axis=1,
        )                                              # [4096, 256]
        wkv_p = np.ascontiguousarray(
            wkvm.reshape(32, 128, 256).transpose(1, 0, 2)
        ).reshape(128, 32 * 256).astype(kvdt)

        # kT: [hd, j] per batch; 4 batches per piece
        ckm = ck[:, :rsp, m, :]                        # [16, rsp, 128]
        kT_p = np.ascontiguousarray(
            ckm.transpose(0, 2, 1).reshape(4, 4, 128, rsp).transpose(0, 2, 1, 3)
        ).reshape(4, 128, 4 * rsp).astype(cdt)

        # v: [j%128, (b,c,d)]; v[b, 128c+p, d] at [p, (b*BCH+c)*128+d]
        cvm = cv[:, :rsp, m, :]                        # [16, rsp, 128]
        v_p = np.ascontiguousarray(
            cvm.reshape(4, 4, BCH, 128, 128).transpose(0, 3, 1, 2, 4)
        ).reshape(4, 128, 4 * rsp).astype(cdt)

        # wo rows for this core: [h, d, n]
        wom = wo[512 * m : 512 * (m + 1), :]           # [512, 4096]
        wo_p = np.ascontiguousarray(
            wom.reshape(NH, 128, DIM)
        ).astype(odt)

        shkT_p = shk[0, :spl, m, :].T                  # [128, spl]
        shv_p = (
            shv[0, :spl, m, :].reshape(spl // 128, 128, 128).transpose(1, 0, 2)
        ).reshape(128, spl)
        cpack = np.concatenate([xT_p, shkT_p, shv_p], axis=1).astype(bdt)

        in_maps.append(
            {
                "cpack": cpack,
                "wq": wq_p,
                "wkv": wkv_p,
                "kT": kT_p,
                "v": v_p,
                "wo": wo_p,
            }
        )
    return in_maps


# ----------------------------------------------------------------------------
# entry point
# ----------------------------------------------------------------------------

_NC_CACHE = {}


def get_nc(spl=512, rsp=1536):
    key = (spl, rsp, CACHE_DT, WKV_DT, WQ_DT, WO_DT, WO_TILES)
    if key not in _NC_CACHE:
        _patch_tile_drain()
        _install_ntff_hook()
        _NC_CACHE[key] = _build_nc(spl, rsp)
    return _NC_CACHE[key]


def prep_inputs(inputs):
    start_pos = int(inputs["start_pos"])
    spl = int(inputs["shared_prefix_length"])
    return _prep_inputs(inputs, spl, start_pos - spl)


def kernel(**inputs):
    from concourse.bass_utils import run_bass_kernel_spmd

    start_pos = int(inputs["start_pos"])
    spl = int(inputs["shared_prefix_length"])
    rsp = start_pos - spl
    nc = get_nc(spl, rsp)
    in_maps = _prep_inputs(inputs, spl, rsp)
    trace = os.environ.get("KERNEL_TRACE", "0") == "1"
    kwargs = {}
    if trace:
        kwargs = dict(
            trace=True,
            trace_cores=list(range(N_CORES)),
        )
    res = run_bass_kernel_spmd(
        nc, in_maps, core_ids=list(range(N_CORES)), **kwargs
    )
    kernel.last_result = res
    y = np.zeros((B, DIM), np.float64)
    for r in res.results:
        y += r["y"].astype(np.float64)
    return y.reshape(B, 1, DIM).astype(np.float32)


# revision 12
# speedup vs baseline: 1.0244x; 1.0244x over previous
"""Trainium2 Bass kernel for sparse (shared-prefix) GQA decode attention.

Full-input contract: kernel(**inputs) takes the unsharded tensors from
setup_inputs() and returns the full [16, 1, 4096] float32 output.

Sharding: tensor-parallel over heads across 8 NeuronCores. Core m owns
query heads 4m..4m+3 and kv head m (GQA group m): wq columns
[512m, 512m+512), wk/wv columns [128m, 128m+128), wo rows [512m, 512m+512),
and head m of the kv caches. Each core computes a partial y_m = attn_m @
wo_m; the host sums the 8 partials (the "all-reduce").

Design notes:
  * kv cache and wk/wv stream in fp8 e3m4 (4-bit mantissa); wq/wo stay
    bf16. Mixed-dtype matmuls (fp8 stationary x bf16 moving) are legal on
    TRN2, so q and the probabilities stay bf16. ~15.6MB/core HBM traffic.
  * RoPE is folded into wq/wk on the host (seqlen=1: one fixed rotation,
    a host-side weight reparameterization): no rope, no transposes on
    device. Projections run weight-stationary and produce qT/xkT/xvT
    directly in [d, batch] orientation.
  * PV runs v-stationary (lhsT = v chunk [j,128d], moving = probs
    [j,4h]); attention lands as attnT [128d, (b,h)] with no transposes.
  * kT and v DMA pieces are interleaved so QK and PV rounds alternate
    per 4-batch group and the PE trails the HBM stream by ~one piece.
  * Output projection packs the PE array 4x with column tiling
    (tile_position=(0,32j)): batch M=16 only fills 1/8 of the array, so
    four n-blocks run concurrently in separate column groups.
  * Input DMAs are split across both HWDGE rings (SP + ACT) in
    consumption order; everything is SBUF-resident.
  * PSUM rule learned on HW: start=True resets has_written for the WHOLE
    bank, so concurrently-accumulating groups live in separate banks.

Problem constants (hardcoded per the harness contract): bsz=16, seqlen=1,
dim=4096, n_heads=32, n_kv=8, hd=128, start_pos=2048,
shared_prefix_length=512 -> rsp=1536, L=2049.
"""

import math
import os
import sys
import types

import numpy as np

# ----------------------------------------------------------------------------
# environment patches (self-contained; no /root/problem reads)
# ----------------------------------------------------------------------------


def _patch_tile_drain():
    """The stock TileContext._drain_and_barrier puts one sem-wait per live
    semaphore on a single Drain instruction; the walrus build in this image
    only accepts a single sync wait per instruction ("Too many sync wait
    commands"). Re-emit the waits as individual EventSemaphore instructions
    on the same sequencer instead."""
    import concourse.tile as tile
    from concourse.vector_clock import ScopedClock

    if getattr(tile.TileContext, "_drain_patched", False):
        return

    def _drain_and_barrier(self, tick_clock, wait_clock):
        nc = self.nc
        drain_inst = nc.sync.drain()
        wait_clock.add_sem_waits(
            drain_inst.ins, ScopedClock({None: tick_clock.global_clock})
        )
        waits = list(drain_inst.ins.sync_info.on_wait)
        if len(waits) > 1:
            by_name = {h.name: h for h in self.sems.allocated().values()}
            try:
                drain_inst.ins.sync_info = None
            except Exception:
                pass
            for w in waits:
                h = by_name.get(w.ant_name)
                assert h is not None, f"no handle for sem {w.ant_name}"
                nc.sync.wait_ge(h, w.wait_value)

        assert self.sems is not None
        popped = nc._tile_sem_poison_stack.pop()
        assert popped is self._sem_poison
        nums = [h.num for h in self.sems.allocated().values()]
        nc._state.prepend_free_semaphores(nums)
        for ps in nc._tile_sem_poison_stack:
            ps.update(nums)

    tile.TileContext._drain_and_barrier = _drain_and_barrier
    tile.TileContext._drain_patched = True


def _install_ntff_hook():
    """Optional: register the axon NTFF profile hook (missing from the
    trimmed antenv package) so trace=True works for profiling, and stub the
    S3 artifact upload (zero-egress container)."""
    try:
        if "antenv.axon_hooks" not in sys.modules:
            mod = types.ModuleType("antenv.axon_hooks")
            mod._hook = None
            mod.set_axon_ntff_profile_hook = lambda h: setattr(mod, "_hook", h)
            mod.get_axon_ntff_profile_hook = lambda: mod._hook
            sys.modules["antenv.axon_hooks"] = mod
            import antenv

            antenv.axon_hooks = mod
            from trn_agent_boot.trn_boot import _ntff_profile_via_ctypes

            mod.set_axon_ntff_profile_hook(
                _ntff_profile_via_ctypes("/opt/axon/libaxon_pjrt.so")
            )
        import concourse.bass_utils as bu

        bu.upload_artifacts = lambda tmpdir: tmpdir
    except Exception:
        pass


def _legalize_multiwait(nc, max_waits=1):
    """This walrus build accepts at most one sync wait per instruction.
    Hoist excess waits into standalone single-wait EventSemaphore
    instructions inserted immediately before, on the same engine."""
    import bass_rust

    uid = 0
    for f in nc.m.functions:
        for bb in f.blocks:
            insts = list(bb.instructions)
            out = []
            changed = False
            for ins in insts:
                si = ins.sync_info
                if si is not None:
                    waits = list(si.on_wait)
                    if len(waits) > max_waits:
                        for w in waits[:-max_waits]:
                            ev = bass_rust.InstEventSemaphore(
                                name=f"{ins.name}_xw{uid}"
                            )
                            uid += 1
                            ev.engine = ins.engine
                            ev.sync_info = bass_rust.SyncInfo(
                                on_wait=[w], on_update=[]
                            )
                            out.append(ev)
                        ins.sync_info = bass_rust.SyncInfo(
                            on_wait=waits[-max_waits:],
                            on_update=list(si.on_update),
                        )
                        changed = True
                out.append(ins)
            if changed:
                bb.instructions = out


# ----------------------------------------------------------------------------
# constants
# ----------------------------------------------------------------------------

N_CORES = 8
B = 16            # batch
DIM = 4096
N_HEADS = 32
N_KV = 8
HD = 128
NH = N_HEADS // N_CORES      # 4 local q heads
R = B * NH                   # 64 cols, r = 4*b + h
SOFTMAX_SCALE = 1.0 / math.sqrt(HD)
WS = 32.0                    # host pre-scale on wk/wv so e3m4 normals cover them

# stream dtypes (env-overridable for A/B tests)
CACHE_DT = os.environ.get("KERNEL_CACHE_DT", "float8e3")
WKV_DT = os.environ.get("KERNEL_WKV_DT", "float8e3")
WQ_DT = os.environ.get("KERNEL_WQ_DT", "bfloat16")
WO_DT = os.environ.get("KERNEL_WO_DT", "bfloat16")

WARMUP_MMS = int(os.environ.get("KERNEL_WARMUP_MMS", "36"))
WO_TILES = int(os.environ.get("KERNEL_WO_TILES", "4"))  # col-tiling width


# ----------------------------------------------------------------------------
# device kernel
# ----------------------------------------------------------------------------


def _build_nc(spl, rsp):
    import concourse.bass as bass
    import concourse.tile as tile
    from concourse import mybir
    from concourse.mybir import ActivationFunctionType as AF

    BF = mybir.dt.bfloat16
    F16 = mybir.dt.float16
    f32 = mybir.dt.float32
    CDT = getattr(mybir.dt, CACHE_DT)
    KVDT = getattr(mybir.dt, WKV_DT)
    QDT = getattr(mybir.dt, WQ_DT)
    ODT = getattr(mybir.dt, WO_DT)

    assert spl % 128 == 0 and rsp % (128 * 4) == 0
    SH_CH = spl // 128           # shared j-chunks (4)
    BCH = rsp // 128             # per-batch cache j-chunks (12)
    NCH = SH_CH + BCH + 1        # total chunks incl. new-token chunk (17)
    NPC = 4                      # batches per kT/v DMA piece
    NPIECE = B // NPC            # 4 pieces

    nc = bass.Bass(
        "TRN2", target_bir_lowering=False, debug=False, num_devices=N_CORES
    )

    cpack_d = nc.dram_tensor("cpack", [128, 32 * B + 2 * spl], BF,
                             kind="ExternalInput").ap()
    wq_d = nc.dram_tensor("wq", [4, 128, 8 * NH * HD], QDT,
                          kind="ExternalInput").ap()
    wkv_d = nc.dram_tensor("wkv", [128, 32 * 2 * HD], KVDT,
                           kind="ExternalInput").ap()
    kT_d = nc.dram_tensor("kT", [NPIECE, 128, NPC * rsp], CDT,
                          kind="ExternalInput").ap()
    v_d = nc.dram_tensor("v", [NPIECE, 128, NPC * rsp], CDT,
                         kind="ExternalInput").ap()
    wo_d = nc.dram_tensor("wo", [NH, 128, DIM], ODT,
                          kind="ExternalInput").ap()
    y_d = nc.dram_tensor("y", [B, DIM], f32, kind="ExternalOutput").ap()

    with tile.TileContext(nc) as tc:
        with tc.tile_pool(name="const", bufs=1) as const:
            # ---------------- resident SBUF tiles ----------------
            ones_sb = const.tile([128, 1], BF, tag="ones")
            ones1p = const.tile([1, 128], BF, tag="ones1p")
            g64 = const.tile([128, R], BF, tag="g64")
            scr1 = const.tile([1, 4], BF, tag="scr1")
            cpack_sb = const.tile([128, 32 * B + 2 * spl], BF, tag="cpack")
            wq_sb = const.tile([128, 32 * NH * HD], QDT, tag="wq")
            wkv_sb = const.tile([128, 32 * 2 * HD], KVDT, tag="wkv")
            kT_sb = const.tile([128, B * rsp], CDT, tag="kT")
            v_sb = const.tile([128, B * rsp], CDT, tag="v")
            wo_sb = const.tile([128, NH * DIM], ODT, tag="wo")
            qT_sb = const.tile([128, R], BF, tag="qT")
            xkT_sb = const.tile([128, B], BF, tag="xkT")
            xvT_sb = const.tile([128, B], BF, tag="xvT")
            xk_bc = const.tile([128, R], BF, tag="xk_bc")
            xv_bc = const.tile([128, R], BF, tag="xv_bc")
            prod_sb = const.tile([128, R], F16, tag="prod")
            pT = const.tile([128, NCH, R], BF, tag="pT")
            sum1 = const.tile([1, R], f32, tag="sum1")
            rinv1 = const.tile([1, R], BF, tag="rinv1")
            rbc_sb = const.tile([128, 2, R], BF, tag="rbc")  # [pnew_bc|rinv_bc]
            attnT = const.tile([128, R], BF, tag="attnT")    # cols 4b+h
            attnT2 = const.tile([128, R], BF, tag="attnT2")  # cols 16h+b
            y_sb = const.tile([128, 2 * 512], f32, tag="y")  # band j=n%4 rows

            xT = cpack_sb[:, : 32 * B]
            shkT = cpack_sb[:, 32 * B : 32 * B + spl]
            shv = cpack_sb[:, 32 * B + spl :]

            # ---------------- constants ----------------
            nc.vector.memset(ones_sb, 1.0)
            nc.vector.memset(ones1p, 1.0)
            nc.vector.memset(g64, 0.0)
            nc.vector.memset(pT[:, NCH - 1, :], 0.0)

            # ---------------- input DMA stream ----------------
            # consumption order: x/shared, wq, wkv, (kT, v interleaved), wo
            # all on the SP HWDGE ring: it keeps strict FIFO order and the
            # stream saturates HBM; ACT-ring issues would block the ACT
            # queue (issue stalls on ring space) and starve the exps.
            WQP = 8 * NH * HD
            KVP = NPC * rsp
            nc.sync.dma_start(out=cpack_sb, in_=cpack_d)
            for g in range(4):
                nc.sync.dma_start(out=wq_sb[:, WQP * g : WQP * (g + 1)],
                                  in_=wq_d[g])
            nc.sync.dma_start(out=wkv_sb, in_=wkv_d)
            for g in range(NPIECE):
                nc.sync.dma_start(out=kT_sb[:, KVP * g : KVP * (g + 1)],
                                  in_=kT_d[g])
                nc.sync.dma_start(out=v_sb[:, KVP * g : KVP * (g + 1)],
                                  in_=v_d[g])
            for h in range(NH):
                nc.sync.dma_start(out=wo_sb[:, DIM * h : DIM * (h + 1)],
                                  in_=wo_d[h])

            # preload the ACT Exp table during the DMA dead time
            nc.scalar.activation(out=scr1, in_=g64[0:1, 0:4], func=AF.Exp)

            # ---------------- PE warmup (HAM clock gate) ----------------
            with tc.tile_pool(name="pwarm", bufs=1, space="PSUM") as pw:
                wps = pw.tile([1, R], f32, tag="wps")
                for i in range(WARMUP_MMS):
                    nc.tensor.matmul(wps, ones_sb, g64, start=True, stop=True)

            # ---------------- long-lived PSUM pools ----------------
            with tc.tile_pool(name="ppv", bufs=1, space="PSUM") as ppvp, \
                 tc.tile_pool(name="psum1", bufs=1, space="PSUM") as psump, \
                 tc.tile_pool(name="pnew", bufs=1, space="PSUM") as pnewp:
                ps_pv = ppvp.tile([128, R], f32, tag="pv")
                ps_sum = psump.tile([1, R], f32, tag="sum")
                ps_snew = pnewp.tile([1, R], f32, tag="snew")
                ps_bc = pnewp.tile([128, 2, R], f32, tag="bc")

                # ---------------- q projection (weight-stationary) --------
                # NB: start=True resets has_written for the WHOLE psum bank,
                # so every concurrently-accumulating group needs its own
                # bank (PSUM pool slots are bank-granular).
                with tc.tile_pool(name="psq", bufs=1, space="PSUM") as psqp:
                    psq = [psqp.tile([128, B], f32, tag=f"psq{h}",
                                     name=f"psq{h}") for h in range(NH)]
                    for kc in range(32):
                        rx = xT[:, B * kc : B * (kc + 1)]
                        for h in range(NH):
                            nc.tensor.matmul(
                                psq[h],
                                wq_sb[:, (kc * NH + h) * HD : (kc * NH + h + 1) * HD],
                                rx,
                                start=(kc == 0),
                                stop=(kc == 31),
                            )
                    qTv = qT_sb.rearrange("p (b h) -> p b h", h=NH)
                    for h in range(NH):
                        nc.vector.tensor_copy(qTv[:, :, h], psq[h])

                # ---------------- shared-prefix scores + PV ----------------
                with tc.tile_pool(name="psh", bufs=1, space="PSUM") as pshp:
                    ps_sh = pshp.tile([128, SH_CH, R], f32, tag="sh")
                    for c in range(SH_CH):
                        nc.tensor.matmul(
                            ps_sh[:, c, :],
                            shkT[:, 128 * c : 128 * (c + 1)],
                            qT_sb,
                            start=True, stop=True,
                        )
                    nc.scalar.activation(
                        out=pT[:, 0:SH_CH, :], in_=ps_sh,
                        func=AF.Exp, scale=SOFTMAX_SCALE,
                    )
                # shared PV opens the big accumulation into ps_pv
                for c in range(SH_CH):
                    nc.tensor.matmul(
                        ps_pv,
                        shv[:, 128 * c : 128 * (c + 1)],
                        pT[:, c, :],
                        start=(c == 0), stop=False,
                        skip_group_check=True,
                    )
                # shared part of the softmax denominators
                for c in range(SH_CH):
                    nc.tensor.matmul(
                        ps_sum, ones_sb, pT[:, c, :],
                        start=(c == 0), stop=False,
                        skip_group_check=True,
                    )

                # ---------------- k/v projection ----------------
                with tc.tile_pool(name="pskv", bufs=1, space="PSUM") as pskvp:
                    pskv = [pskvp.tile([128, B], f32, tag=f"pskv{u}",
                                       name=f"pskv{u}") for u in range(2)]
                    for kc in range(32):
                        rx = xT[:, B * kc : B * (kc + 1)]
                        for u in range(2):
                            nc.tensor.matmul(
                                pskv[u],
                                wkv_sb[:, (kc * 2 + u) * HD : (kc * 2 + u + 1) * HD],
                                rx,
                                start=(kc == 0),
                                stop=(kc == 31),
                            )
                    nc.scalar.activation(out=xkT_sb, in_=pskv[0],
                                         func=AF.Copy, scale=1.0 / WS)
                    nc.scalar.activation(out=xvT_sb, in_=pskv[1],
                                         func=AF.Copy, scale=1.0 / WS)

                # new-token k/v broadcast + q.k product (DVE)
                xkv_ = xk_bc.rearrange("p (b h) -> p b h", h=NH)
                xvv_ = xv_bc.rearrange("p (b h) -> p b h", h=NH)
                for h in range(NH):
                    nc.vector.tensor_copy(xkv_[:, :, h], xkT_sb)
                    nc.vector.tensor_copy(xvv_[:, :, h], xvT_sb)
                nc.vector.tensor_mul(prod_sb, qT_sb, xk_bc)

                # ---------------- interleaved QK / PV rounds ----------------
                pTc = pT[:, SH_CH : SH_CH + BCH, :].rearrange(
                    "p c (g r2) -> p c g r2", r2=2 * NH
                )

                def pv_piece(g):
                    for j in range(NPC):
                        b = NPC * g + j
                        for c in range(BCH):
                            nc.tensor.matmul(
                                ps_pv[:, NH * b : NH * (b + 1)],
                                v_sb[:, rsp * b + 128 * c : rsp * b + 128 * (c + 1)],
                                pT[:, SH_CH + c, NH * b : NH * (b + 1)],
                                start=False, stop=(c == BCH - 1),
                                skip_group_check=True,
                            )

                with tc.tile_pool(name="pqk", bufs=3, space="PSUM") as pqkp:
                    for rnd in range(NPIECE):
                        for g2 in range(2):     # 2 batches per psum tile
                            grp = 2 * rnd + g2
                            qk = pqkp.tile([128, BCH, 2 * NH], f32, tag="qk",
                                           name=f"qk{grp}")
                            for b2 in range(2):
                                b = 2 * grp + b2
                                rq = qT_sb[:, NH * b : NH * (b + 1)]
                                for c in range(BCH):
                                    nc.tensor.matmul(
                                        qk[:, c, NH * b2 : NH * (b2 + 1)],
                                        kT_sb[:, rsp * b + 128 * c : rsp * b + 128 * (c + 1)],
                                        rq,
                                        start=True, stop=True,
                                    )
                            nc.scalar.activation(
                                out=pTc[:, :, grp, :], in_=qk,
                                func=AF.Exp, scale=SOFTMAX_SCALE,
                            )
                        if rnd == 0:
                            # new-token score + prob (fits the round-0 gap)
                            nc.tensor.matmul(ps_snew, ones_sb, prod_sb,
                                             start=True, stop=True)
                            nc.scalar.activation(
                                out=pT[0:1, NCH - 1, :], in_=ps_snew,
                                func=AF.Exp, scale=SOFTMAX_SCALE,
                            )
                        if rnd == 1:
                            nc.tensor.matmul(ps_bc[:, 0, :], ones1p,
                                             pT[0:1, NCH - 1, :],
                                             start=True, stop=True)
                        if rnd == NPIECE - 1:
                            # denominators: cache chunks + new-token chunk
                            for c in range(BCH):
                                nc.tensor.matmul(
                                    ps_sum, ones_sb, pT[:, SH_CH + c, :],
                                    start=False, stop=False,
                                    skip_group_check=True,
                                )
                            nc.tensor.matmul(
                                ps_sum, ones_sb, pT[:, NCH - 1, :],
                                start=False, stop=True, skip_group_check=True,
                            )
                            nc.vector.tensor_copy(sum1, ps_sum)
                            with nc.allow_low_precision(reason="1/rowsum"):
                                nc.vector.reciprocal(rinv1, sum1)
                        pv_piece(rnd)
                        # per-piece evacuation of the attention output
                        S = slice(NH * NPC * rnd, NH * NPC * (rnd + 1))
                        nc.scalar.activation(out=attnT[:, S],
                                             in_=ps_pv[:, S], func=AF.Copy)

                # 1/rowsum broadcast, new-token add, normalize, reorder
                nc.tensor.matmul(ps_bc[:, 1, :], ones1p, rinv1,
                                 start=True, stop=True)
                nc.vector.tensor_copy(rbc_sb, ps_bc)
                nc.vector.tensor_mul(xv_bc, xv_bc, rbc_sb[:, 0, :])
                nc.vector.tensor_add(attnT, attnT, xv_bc)
                a2v = attnT2.rearrange("p (h b) -> p b h", b=B)
                nc.vector.tensor_mul(
                    a2v,
                    attnT.rearrange("p (b h) -> p b h", h=NH),
                    rbc_sb[:, 1, :].rearrange("p (b h) -> p b h", h=NH),
                )

            # ---------------- output projection (col-tiled) ----------------
            # M=16 fills 1/8 of the PE array; run WO_TILES n-blocks
            # concurrently in separate 32-column groups of the array.
            NQ = 8 // WO_TILES          # quads per h-pass
            with tc.tile_pool(name="py", bufs=1, space="PSUM") as pyp:
                ys = [pyp.tile([128, 512], f32, tag=f"y{q}", name=f"ys{q}")
                      for q in range(NQ)]
                for h in range(NH):
                    lq = attnT2[:, B * h : B * (h + 1)]
                    for q in range(NQ):
                        for j in range(WO_TILES):
                            n = WO_TILES * q + j
                            nc.tensor.matmul(
                                ys[q][32 * j : 32 * j + B, :],
                                lq,
                                wo_sb[:, DIM * h + 512 * n : DIM * h + 512 * (n + 1)],
                                start=(h == 0),
                                stop=(h == NH - 1),
                                tile_position=(0, 32 * j),
                            )
                # evacuate bands (DVE/ACT alternating), then one DMA per
                # 32-partition band with a strided out AP over its n-blocks
                y_dv = y_d.rearrange("b (q j n) -> b q j n", q=NQ, j=WO_TILES)
                for j in range(WO_TILES):
                    band = slice(32 * j, 32 * j + B)
                    for q in range(NQ):
                        if j % 2 == 0:
                            nc.vector.tensor_copy(
                                y_sb[band, 512 * q : 512 * (q + 1)],
                                ys[q][band, :],
                            )
                        else:
                            nc.scalar.activation(
                                out=y_sb[band, 512 * q : 512 * (q + 1)],
                                in_=ys[q][band, :], func=AF.Copy,
                            )
                    nc.sync.dma_start(
                        out=y_dv[:, :, j, :], in_=y_sb[band, :]
                    )

            if os.environ.get("KERNEL_DEBUG") == "1":
                def dbg(name, ap):
                    d = nc.dram_tensor(
                        f"dbg_{name}", list(ap.shape), ap.dtype,
                        kind="ExternalOutput",
                    ).ap()
                    nc.sync.dma_start(out=d, in_=ap)
                dbg("qT", qT_sb)
                dbg("xkT", xkT_sb)
                dbg("xvT", xvT_sb)
                dbg("pT", pT)
                dbg("sum1", sum1)
                dbg("rbc", rbc_sb)
                dbg("attnT", attnT)
                dbg("attnT2", attnT2)
                dbg("prod", prod_sb)

    if os.environ.get("KERNEL_SKIP_LEGALIZE") != "1":
        _legalize_multiwait(nc)
    return nc


# ----------------------------------------------------------------------------
# host-side sharding / layout prep
# ----------------------------------------------------------------------------


def _np_dt(name):
    import ml_dtypes

    return {
        "bfloat16": ml_dtypes.bfloat16,
        "float8e3": ml_dtypes.float8_e3m4,
        "float16": np.float16,
        "float32": np.float32,
    }[name]


def _prep_inputs(inputs, spl, rsp):
    x = np.asarray(inputs["x"], np.float32)            # [16, 1, 4096]
    wq = np.asarray(inputs["wq"], np.float32)
    wk = np.asarray(inputs["wk"], np.float32)
    wv = np.asarray(inputs["wv"], np.float32)
    wo = np.asarray(inputs["wo"], np.float32)
    ck = np.asarray(inputs["cache_k"], np.float32)     # [16, 4096, 8, 128]
    cv = np.asarray(inputs["cache_v"], np.float32)
    shk = np.asarray(inputs["shared_cache_k"], np.float32)  # [1, 512, 8, 128]
    shv = np.asarray(inputs["shared_cache_v"], np.float32)
    cos = np.asarray(inputs["freqs_cos"], np.float32)[0]    # [64]
    sin = np.asarray(inputs["freqs_sin"], np.float32)[0]

    bdt = _np_dt("bfloat16")
    cdt = _np_dt(CACHE_DT)
    kvdt = _np_dt(WKV_DT)
    qdt = _np_dt(WQ_DT)
    odt = _np_dt(WO_DT)

    def fold_rope(w):
        # seqlen=1 decode: rope is one fixed pairwise rotation; fold it
        # into the projection columns (a host-side reparameterization)
        W = w.reshape(w.shape[0], -1, 64, 2)
        we, wo_ = W[..., 0], W[..., 1]
        return np.stack(
            [we * cos - wo_ * sin, we * sin + wo_ * cos], -1
        ).reshape(w.shape)

    wq_r = fold_rope(wq)
    wk_r = fold_rope(wk) * WS
    wv_s = wv * WS

    xm = x[:, 0, :]                                    # [16, 4096]
    xT_p = np.ascontiguousarray(
        xm.T.reshape(32, 128, B).transpose(1, 0, 2)
    ).reshape(128, 32 * B)

    BCH = rsp // 128
    in_maps = []
    for m in range(N_CORES):
        # wq': pieces of 8 k-chunks; col ((kc%8)*4+h)*128+d
        wqm = wq_r[:, 512 * m : 512 * (m + 1)]         # [4096, 512]
        wq_p = np.ascontiguousarray(
            wqm.reshape(4, 8, 128, NH * HD).transpose(0, 2, 1, 3)
        ).reshape(4, 128, 8 * NH * HD).astype(qdt)

        # wkv': col (kc*2+u)*128+d
        wkvm = np.concatenate(
            [wk_r[:, 128 * m : 128 * (m + 1)], wv_s[:, 128 * m : 128 * (m + 1)]],
            axis=1,
        )                                              # [4096, 256]
        wkv_p = np.ascontiguousarray(
            wkvm.reshape(32, 128, 256).transpose(1, 0, 2)
        ).reshape(128, 32 * 256).astype(kvdt)

        # kT: [hd, j] per batch; 4 batches per piece
        ckm = ck[:, :rsp, m, :]                        # [16, rsp, 128]
        kT_p = np.ascontiguousarray(
            ckm.transpose(0, 2, 1).reshape(4, 4, 128, rsp).transpose(0, 2, 1, 3)
        ).reshape(4, 128, 4 * rsp).astype(cdt)

        # v: [j%128, (b,c,d)]; v[b, 128c+p, d] at [p, (b*BCH+c)*128+d]
        cvm = cv[:, :rsp, m, :]                        # [16, rsp, 128]
        v_p = np.ascontiguousarray(
            cvm.reshape(4, 4, BCH, 128, 128).transpose(0, 3, 1, 2, 4)
        ).reshape(4, 128, 4 * rsp).astype(cdt)

        # wo rows for this core: [h, d, n]
        wom = wo[512 * m : 512 * (m + 1), :]           # [512, 4096]
        wo_p = np.ascontiguousarray(
            wom.reshape(NH, 128, DIM)
        ).astype(odt)

        shkT_p = shk[0, :spl, m, :].T                  # [128, spl]
        shv_p = (
            shv[0, :spl, m, :].reshape(spl // 128, 128, 128).transpose(1, 0, 2)
        ).reshape(128, spl)
        cpack = np.concatenate([xT_p, shkT_p, shv_p], axis=1).astype(bdt)

        in_maps.append(
            {
                "cpack": cpack,
                "wq": wq_p,
                "wkv": wkv_p,
                "kT": kT_p,
                "v": v_p,
                "wo": wo_p,
            }
        )
    return in_maps


# ----------------------------------------------------------------------------
# entry point
# ----------------------------------------------------------------------------

_NC_CACHE = {}


def get_nc(spl=512, rsp=1536):
    key = (spl, rsp, CACHE_DT, WKV_DT, WQ_DT, WO_DT, WO_TILES)
    if key not in _NC_CACHE:
        _patch_tile_drain()
        _install_ntff_hook()
        _NC_CACHE[key] = _build_nc(spl, rsp)
    return _NC_CACHE[key]


def prep_inputs(inputs):
    start_pos = int(inputs["start_pos"])
    spl = int(inputs["shared_prefix_length"])
    return _prep_inputs(inputs, spl, start_pos - spl)


def kernel(**inputs):
    from concourse.bass_utils import run_bass_kernel_spmd

    start_pos = int(inputs["start_pos"])
    spl = int(inputs["shared_prefix_length"])
    rsp = start_pos - spl
    nc = get_nc(spl, rsp)
    in_maps = _prep_inputs(inputs, spl, rsp)
    trace = os.environ.get("KERNEL_TRACE", "0") == "1"
    kwargs = {}
    if trace:
        kwargs = dict(
            trace=True,
            trace_cores=list(range(N_CORES)),
        )
    res = run_bass_kernel_spmd(
        nc, in_maps, core_ids=list(range(N_CORES)), **kwargs
    )
    kernel.last_result = res
    y = np.zeros((B, DIM), np.float64)
    for r in res.results:
        y += r["y"].astype(np.float64)
    return y.reshape(B, 1, DIM).astype(np.float32)


# revision 13
# speedup vs baseline: 1.0855x; 1.0597x over previous
"""Trainium2 Bass kernel for sparse (shared-prefix) GQA decode attention.

Full-input contract: kernel(**inputs) takes the unsharded tensors from
setup_inputs() and returns the full [16, 1, 4096] float32 output.

Sharding: tensor-parallel over heads across 8 NeuronCores. Core m owns
query heads 4m..4m+3 and kv head m (GQA group m): wq columns
[512m, 512m+512), wk/wv columns [128m, 128m+128), wo rows [512m, 512m+512),
and head m of the kv caches. Each core computes a partial y_m = attn_m @
wo_m; the host sums the 8 partials (the "all-reduce").

Design notes:
  * kv cache and wk/wv stream in fp8 e3m4 (4-bit mantissa); wq/wo stay
    bf16. Mixed-dtype matmuls (fp8 stationary x bf16 moving) are legal on
    TRN2, so q and the probabilities stay bf16. ~15.6MB/core HBM traffic.
  * RoPE is folded into wq/wk on the host (seqlen=1: one fixed rotation,
    a host-side weight reparameterization): no rope, no transposes on
    device. Projections run weight-stationary and produce qT/xkT/xvT
    directly in [d, batch] orientation.
  * PV runs v-stationary (lhsT = v chunk [j,128d], moving = probs
    [j,4h]); attention lands as attnT [128d, (b,h)] with no transposes.
  * kT and v DMA pieces are interleaved so QK and PV rounds alternate
    per 4-batch group and the PE trails the HBM stream by ~one piece.
  * Output projection packs the PE array 4x with column tiling
    (tile_position=(0,32j)): batch M=16 only fills 1/8 of the array, so
    four n-blocks run concurrently in separate column groups.
  * Input DMAs are split across both HWDGE rings (SP + ACT) in
    consumption order; everything is SBUF-resident.
  * PSUM rule learned on HW: start=True resets has_written for the WHOLE
    bank, so concurrently-accumulating groups live in separate banks.

Problem constants (hardcoded per the harness contract): bsz=16, seqlen=1,
dim=4096, n_heads=32, n_kv=8, hd=128, start_pos=2048,
shared_prefix_length=512 -> rsp=1536, L=2049.
"""

import math
import os
import sys
import types

import numpy as np

# ----------------------------------------------------------------------------
# environment patches (self-contained; no /root/problem reads)
# ----------------------------------------------------------------------------


def _patch_tile_drain():
    """The stock TileContext._drain_and_barrier puts one sem-wait per live
    semaphore on a single Drain instruction; the walrus build in this image
    only accepts a single sync wait per instruction ("Too many sync wait
    commands"). Re-emit the waits as individual EventSemaphore instructions
    on the same sequencer instead."""
    import concourse.tile as tile
    from concourse.vector_clock import ScopedClock

    if getattr(tile.TileContext, "_drain_patched", False):
        return

    def _drain_and_barrier(self, tick_clock, wait_clock):
        nc = self.nc
        drain_inst = nc.sync.drain()
        wait_clock.add_sem_waits(
            drain_inst.ins, ScopedClock({None: tick_clock.global_clock})
        )
        waits = list(drain_inst.ins.sync_info.on_wait)
        if len(waits) > 1:
            by_name = {h.name: h for h in self.sems.allocated().values()}
            try:
                drain_inst.ins.sync_info = None
            except Exception:
                pass
            for w in waits:
                h = by_name.get(w.ant_name)
                assert h is not None, f"no handle for sem {w.ant_name}"
                nc.sync.wait_ge(h, w.wait_value)

        assert self.sems is not None
        popped = nc._tile_sem_poison_stack.pop()
        assert popped is self._sem_poison
        nums = [h.num for h in self.sems.allocated().values()]
        nc._state.prepend_free_semaphores(nums)
        for ps in nc._tile_sem_poison_stack:
            ps.update(nums)

    tile.TileContext._drain_and_barrier = _drain_and_barrier
    tile.TileContext._drain_patched = True


def _install_ntff_hook():
    """Optional: register the axon NTFF profile hook (missing from the
    trimmed antenv package) so trace=True works for profiling, and stub the
    S3 artifact upload (zero-egress container)."""
    try:
        if "antenv.axon_hooks" not in sys.modules:
            mod = types.ModuleType("antenv.axon_hooks")
            mod._hook = None
            mod.set_axon_ntff_profile_hook = lambda h: setattr(mod, "_hook", h)
            mod.get_axon_ntff_profile_hook = lambda: mod._hook
            sys.modules["antenv.axon_hooks"] = mod
            import antenv

            antenv.axon_hooks = mod
            from trn_agent_boot.trn_boot import _ntff_profile_via_ctypes

            mod.set_axon_ntff_profile_hook(
                _ntff_profile_via_ctypes("/opt/axon/libaxon_pjrt.so")
            )
        import concourse.bass_utils as bu

        bu.upload_artifacts = lambda tmpdir: tmpdir
    except Exception:
        pass


def _legalize_multiwait(nc, max_waits=1):
    """This walrus build accepts at most one sync wait per instruction.
    Hoist excess waits into standalone single-wait EventSemaphore
    instructions inserted immediately before, on the same engine."""
    import bass_rust

    uid = 0
    for f in nc.m.functions:
        for bb in f.blocks:
            insts = list(bb.instructions)
            out = []
            changed = False
            for ins in insts:
                si = ins.sync_info
                if si is not None:
                    waits = list(si.on_wait)
                    if len(waits) > max_waits:
                        for w in waits[:-max_waits]:
                            ev = bass_rust.InstEventSemaphore(
                                name=f"{ins.name}_xw{uid}"
                            )
                            uid += 1
                            ev.engine = ins.engine
                            ev.sync_info = bass_rust.SyncInfo(
                                on_wait=[w], on_update=[]
                            )
                            out.append(ev)
                        ins.sync_info = bass_rust.SyncInfo(
                            on_wait=waits[-max_waits:],
                            on_update=list(si.on_update),
                        )
                        changed = True
                out.append(ins)
            if changed:
                bb.instructions = out


# ----------------------------------------------------------------------------
# constants
# ----------------------------------------------------------------------------

N_CORES = 8
B = 16            # batch
DIM = 4096
N_HEADS = 32
N_KV = 8
HD = 128
NH = N_HEADS // N_CORES      # 4 local q heads
R = B * NH                   # 64 cols, r = 4*b + h
SOFTMAX_SCALE = 1.0 / math.sqrt(HD)
WS = 32.0                    # host pre-scale on wk/wv so e3m4 normals cover them

# stream dtypes (env-overridable for A/B tests)
CACHE_DT = os.environ.get("KERNEL_CACHE_DT", "float8e3")
WKV_DT = os.environ.get("KERNEL_WKV_DT", "float8e3")
WQ_DT = os.environ.get("KERNEL_WQ_DT", "bfloat16")
WO_DT = os.environ.get("KERNEL_WO_DT", "bfloat16")

WARMUP_MMS = int(os.environ.get("KERNEL_WARMUP_MMS", "36"))
WO_TILES = int(os.environ.get("KERNEL_WO_TILES", "4"))  # col-tiling width


# ----------------------------------------------------------------------------
# device kernel
# ----------------------------------------------------------------------------


def _build_nc(spl, rsp):
    import concourse.bass as bass
    import concourse.tile as tile
    from concourse import mybir
    from concourse.mybir import ActivationFunctionType as AF

    BF = mybir.dt.bfloat16
    F16 = mybir.dt.float16
    f32 = mybir.dt.float32
    CDT = getattr(mybir.dt, CACHE_DT)
    KVDT = getattr(mybir.dt, WKV_DT)
    QDT = getattr(mybir.dt, WQ_DT)
    ODT = getattr(mybir.dt, WO_DT)

    assert spl % 128 == 0 and rsp % (128 * 4) == 0
    SH_CH = spl // 128           # shared j-chunks (4)
    BCH = rsp // 128             # per-batch cache j-chunks (12)
    NCH = SH_CH + BCH + 1        # total chunks incl. new-token chunk (17)
    NPC = 4                      # batches per kT/v DMA piece
    NPIECE = B // NPC            # 4 pieces

    nc = bass.Bass(
        "TRN2", target_bir_lowering=False, debug=False, num_devices=N_CORES
    )

    cpack_d = nc.dram_tensor("cpack", [128, 32 * B + 2 * spl], BF,
                             kind="ExternalInput").ap()
    wq_d = nc.dram_tensor("wq", [4, 128, 8 * NH * HD], QDT,
                          kind="ExternalInput").ap()
    wkv_d = nc.dram_tensor("wkv", [128, 32 * 2 * HD], KVDT,
                           kind="ExternalInput").ap()
    kT_d = nc.dram_tensor("kT", [NPIECE, 128, NPC * rsp], CDT,
                          kind="ExternalInput").ap()
    v_d = nc.dram_tensor("v", [NPIECE, 128, NPC * rsp], CDT,
                         kind="ExternalInput").ap()
    wo_d = nc.dram_tensor("wo", [NH, 128, DIM], ODT,
                          kind="ExternalInput").ap()
    y_d = nc.dram_tensor("y", [B, DIM], f32, kind="ExternalOutput").ap()

    with tile.TileContext(nc) as tc:
        with tc.tile_pool(name="const", bufs=1) as const:
            # ---------------- resident SBUF tiles ----------------
            ones_sb = const.tile([128, 1], BF, tag="ones")
            ones1p = const.tile([1, 128], BF, tag="ones1p")
            g64 = const.tile([128, R], BF, tag="g64")
            scr1 = const.tile([1, 4], BF, tag="scr1")
            cpack_sb = const.tile([128, 32 * B + 2 * spl], BF, tag="cpack")
            wq_sb = const.tile([128, 32 * NH * HD], QDT, tag="wq")
            wkv_sb = const.tile([128, 32 * 2 * HD], KVDT, tag="wkv")
            kT_sb = const.tile([128, B * rsp], CDT, tag="kT")
            v_sb = const.tile([128, B * rsp], CDT, tag="v")
            wo_sb = const.tile([128, NH * DIM], ODT, tag="wo")
            qT_sb = const.tile([128, R], BF, tag="qT")
            xkT_sb = const.tile([128, B], BF, tag="xkT")
            xvT_sb = const.tile([128, B], BF, tag="xvT")
            xk_bc = const.tile([128, R], BF, tag="xk_bc")
            xv_bc = const.tile([128, R], BF, tag="xv_bc")
            prod_sb = const.tile([128, R], F16, tag="prod")
            pT = const.tile([128, NCH, R], BF, tag="pT")
            sum1 = const.tile([1, R], f32, tag="sum1")
            rinv1 = const.tile([1, R], BF, tag="rinv1")
            rbc_sb = const.tile([128, 2, R], BF, tag="rbc")  # [pnew_bc|rinv_bc]
            attnT = const.tile([128, R], BF, tag="attnT")    # cols 4b+h
            attnT2 = const.tile([128, R], BF, tag="attnT2")  # cols 16h+b
            y_sb = const.tile([128, 2 * 512], f32, tag="y")  # band j=n%4 rows

            xT = cpack_sb[:, : 32 * B]
            shkT = cpack_sb[:, 32 * B : 32 * B + spl]
            shv = cpack_sb[:, 32 * B + spl :]

            # ---------------- constants ----------------
            nc.vector.memset(ones_sb, 1.0)
            nc.vector.memset(ones1p, 1.0)
            nc.vector.memset(g64, 0.0)
            nc.vector.memset(pT[:, NCH - 1, :], 0.0)

            # ---------------- input DMA stream ----------------
            # consumption order: x/shared, wq, wkv, (kT, v interleaved), wo
            # all on the SP HWDGE ring: it keeps strict FIFO order and the
            # stream saturates HBM; ACT-ring issues would block the ACT
            # queue (issue stalls on ring space) and starve the exps.
            WQP = 8 * NH * HD
            KVP = NPC * rsp
            nc.sync.dma_start(out=cpack_sb, in_=cpack_d)
            for g in range(4):
                nc.sync.dma_start(out=wq_sb[:, WQP * g : WQP * (g + 1)],
                                  in_=wq_d[g])
            nc.sync.dma_start(out=wkv_sb, in_=wkv_d)
            for g in range(NPIECE):
                nc.sync.dma_start(out=kT_sb[:, KVP * g : KVP * (g + 1)],
                                  in_=kT_d[g])
                nc.sync.dma_start(out=v_sb[:, KVP * g : KVP * (g + 1)],
                                  in_=v_d[g])
            for h in range(NH):
                nc.sync.dma_start(out=wo_sb[:, DIM * h : DIM * (h + 1)],
                                  in_=wo_d[h])

            # preload the ACT Exp table during the DMA dead time
            nc.scalar.activation(out=scr1, in_=g64[0:1, 0:4], func=AF.Exp)

            # ---------------- PE warmup (HAM clock gate) ----------------
            with tc.tile_pool(name="pwarm", bufs=1, space="PSUM") as pw:
                wps = pw.tile([1, R], f32, tag="wps")
                for i in range(WARMUP_MMS):
                    nc.tensor.matmul(wps, ones_sb, g64, start=True, stop=True)

            # ---------------- long-lived PSUM pools ----------------
            with tc.tile_pool(name="ppv", bufs=1, space="PSUM") as ppvp, \
                 tc.tile_pool(name="psum1", bufs=1, space="PSUM") as psump, \
                 tc.tile_pool(name="pnew", bufs=1, space="PSUM") as pnewp:
                ps_pv = ppvp.tile([128, R], f32, tag="pv")
                ps_sum = psump.tile([1, R], f32, tag="sum")
                ps_snew = pnewp.tile([1, R], f32, tag="snew")
                ps_bc = pnewp.tile([128, 2, R], f32, tag="bc")

                # ---------------- q projection (weight-stationary) --------
                # NB: start=True resets has_written for the WHOLE psum bank,
                # so every concurrently-accumulating group needs its own
                # bank (PSUM pool slots are bank-granular).
                with tc.tile_pool(name="psq", bufs=1, space="PSUM") as psqp:
                    psq = [psqp.tile([128, B], f32, tag=f"psq{h}",
                                     name=f"psq{h}") for h in range(NH)]
                    for kc in range(32):
                        rx = xT[:, B * kc : B * (kc + 1)]
                        for h in range(NH):
                            nc.tensor.matmul(
                                psq[h],
                                wq_sb[:, (kc * NH + h) * HD : (kc * NH + h + 1) * HD],
                                rx,
                                start=(kc == 0),
                                stop=(kc == 31),
                            )
                    qTv = qT_sb.rearrange("p (b h) -> p b h", h=NH)
                    for h in range(NH):
                        nc.vector.tensor_copy(qTv[:, :, h], psq[h])

                # ---------------- shared-prefix scores + PV ----------------
                with tc.tile_pool(name="psh", bufs=1, space="PSUM") as pshp:
                    ps_sh = pshp.tile([128, SH_CH, R], f32, tag="sh")
                    for c in range(SH_CH):
                        nc.tensor.matmul(
                            ps_sh[:, c, :],
                            shkT[:, 128 * c : 128 * (c + 1)],
                            qT_sb,
                            start=True, stop=True,
                        )
                    nc.scalar.activation(
                        out=pT[:, 0:SH_CH, :], in_=ps_sh,
                        func=AF.Exp, scale=SOFTMAX_SCALE,
                    )
                # shared PV opens the big accumulation into ps_pv
                for c in range(SH_CH):
                    nc.tensor.matmul(
                        ps_pv,
                        shv[:, 128 * c : 128 * (c + 1)],
                        pT[:, c, :],
                        start=(c == 0), stop=False,
                        skip_group_check=True,
                    )
                # shared part of the softmax denominators
                for c in range(SH_CH):
                    nc.tensor.matmul(
                        ps_sum, ones_sb, pT[:, c, :],
                        start=(c == 0), stop=False,
                        skip_group_check=True,
                    )

                # ---------------- k/v projection ----------------
                with tc.tile_pool(name="pskv", bufs=1, space="PSUM") as pskvp:
                    pskv = [pskvp.tile([128, B], f32, tag=f"pskv{u}",
                                       name=f"pskv{u}") for u in range(2)]
                    for kc in range(32):
                        rx = xT[:, B * kc : B * (kc + 1)]
                        for u in range(2):
                            nc.tensor.matmul(
                                pskv[u],
                                wkv_sb[:, (kc * 2 + u) * HD : (kc * 2 + u + 1) * HD],
                                rx,
                                start=(kc == 0),
                                stop=(kc == 31),
                            )
                    nc.scalar.activation(out=xkT_sb, in_=pskv[0],
                                         func=AF.Copy, scale=1.0 / WS)
                    nc.scalar.activation(out=xvT_sb, in_=pskv[1],
                                         func=AF.Copy, scale=1.0 / WS)

                # new-token k/v broadcast + q.k product (DVE)
                xkv_ = xk_bc.rearrange("p (b h) -> p b h", h=NH)
                xvv_ = xv_bc.rearrange("p (b h) -> p b h", h=NH)
                for h in range(NH):
                    nc.vector.tensor_copy(xkv_[:, :, h], xkT_sb)
                    nc.vector.tensor_copy(xvv_[:, :, h], xvT_sb)
                nc.vector.tensor_mul(prod_sb, qT_sb, xk_bc)

                # ---------------- per-batch cache scores ----------------
                pTc = pT[:, SH_CH : SH_CH + BCH, :].rearrange(
                    "p c (g r2) -> p c g r2", r2=2 * NH
                )
                with tc.tile_pool(name="pqk", bufs=3, space="PSUM") as pqkp:
                    for grp in range(B // 2):   # 2 batches per psum tile
                        qk = pqkp.tile([128, BCH, 2 * NH], f32, tag="qk",
                                       name=f"qk{grp}")
                        for b2 in range(2):
                            b = 2 * grp + b2
                            rq = qT_sb[:, NH * b : NH * (b + 1)]
                            for c in range(BCH):
                                nc.tensor.matmul(
                                    qk[:, c, NH * b2 : NH * (b2 + 1)],
                                    kT_sb[:, rsp * b + 128 * c : rsp * b + 128 * (c + 1)],
                                    rq,
                                    start=True, stop=True,
                                )
                        nc.scalar.activation(
                            out=pTc[:, :, grp, :], in_=qk,
                            func=AF.Exp, scale=SOFTMAX_SCALE,
                        )
                        if grp == 0:
                            # new-token score + prob (fits the early gap)
                            nc.tensor.matmul(ps_snew, ones_sb, prod_sb,
                                             start=True, stop=True)
                            nc.scalar.activation(
                                out=pT[0:1, NCH - 1, :], in_=ps_snew,
                                func=AF.Exp, scale=SOFTMAX_SCALE,
                            )
                        if grp == 1:
                            nc.tensor.matmul(ps_bc[:, 0, :], ones1p,
                                             pT[0:1, NCH - 1, :],
                                             start=True, stop=True)

                def pv_piece(g):
                    for j in range(NPC):
                        b = NPC * g + j
                        for c in range(BCH):
                            nc.tensor.matmul(
                                ps_pv[:, NH * b : NH * (b + 1)],
                                v_sb[:, rsp * b + 128 * c : rsp * b + 128 * (c + 1)],
                                pT[:, SH_CH + c, NH * b : NH * (b + 1)],
                                start=False, stop=(c == BCH - 1),
                                skip_group_check=True,
                            )

                def evac_piece(g):
                    S = slice(NH * NPC * g, NH * NPC * (g + 1))
                    nc.scalar.activation(out=attnT[:, S], in_=ps_pv[:, S],
                                         func=AF.Copy)

                # ---------------- PV + denominator chain ----------------
                pv_piece(0)
                evac_piece(0)
                # denominators: cache chunks + new-token chunk
                for c in range(BCH):
                    nc.tensor.matmul(
                        ps_sum, ones_sb, pT[:, SH_CH + c, :],
                        start=False, stop=False, skip_group_check=True,
                    )
                nc.tensor.matmul(
                    ps_sum, ones_sb, pT[:, NCH - 1, :],
                    start=False, stop=True, skip_group_check=True,
                )
                nc.vector.tensor_copy(sum1, ps_sum)
                with nc.allow_low_precision(reason="1/rowsum"):
                    nc.vector.reciprocal(rinv1, sum1)
                pv_piece(1)
                evac_piece(1)
                nc.tensor.matmul(ps_bc[:, 1, :], ones1p, rinv1,
                                 start=True, stop=True)
                pv_piece(2)
                evac_piece(2)
                pv_piece(3)
                evac_piece(3)

                # 1/rowsum broadcast, new-token add, normalize, reorder
                nc.vector.tensor_copy(rbc_sb, ps_bc)
                nc.vector.tensor_mul(xv_bc, xv_bc, rbc_sb[:, 0, :])
                nc.vector.tensor_add(attnT, attnT, xv_bc)
                a2v = attnT2.rearrange("p (h b) -> p b h", b=B)
                nc.vector.tensor_mul(
                    a2v,
                    attnT.rearrange("p (b h) -> p b h", h=NH),
                    rbc_sb[:, 1, :].rearrange("p (b h) -> p b h", h=NH),
                )

            # ---------------- output projection (col-tiled) ----------------
            # M=16 fills 1/8 of the PE array; run WO_TILES n-blocks
            # concurrently in separate 32-column groups of the array.
            NQ = 8 // WO_TILES          # quads per h-pass
            with tc.tile_pool(name="py", bufs=1, space="PSUM") as pyp:
                ys = [pyp.tile([128, 512], f32, tag=f"y{q}", name=f"ys{q}")
                      for q in range(NQ)]
                for h in range(NH):
                    lq = attnT2[:, B * h : B * (h + 1)]
                    for q in range(NQ):
                        for j in range(WO_TILES):
                            n = WO_TILES * q + j
                            nc.tensor.matmul(
                                ys[q][32 * j : 32 * j + B, :],
                                lq,
                                wo_sb[:, DIM * h + 512 * n : DIM * h + 512 * (n + 1)],
                                start=(h == 0),
                                stop=(h == NH - 1),
                                tile_position=(0, 32 * j),
                            )
                # evacuate bands (DVE/ACT alternating), then one DMA per
                # 32-partition band with a strided out AP over its n-blocks
                y_dv = y_d.rearrange("b (q j n) -> b q j n", q=NQ, j=WO_TILES)
                for j in range(WO_TILES):
                    band = slice(32 * j, 32 * j + B)
                    for q in range(NQ):
                        if j % 2 == 0:
                            nc.vector.tensor_copy(
                                y_sb[band, 512 * q : 512 * (q + 1)],
                                ys[q][band, :],
                            )
                        else:
                            nc.scalar.activation(
                                out=y_sb[band, 512 * q : 512 * (q + 1)],
                                in_=ys[q][band, :], func=AF.Copy,
                            )
                    nc.sync.dma_start(
                        out=y_dv[:, :, j, :], in_=y_sb[band, :]
                    )

            if os.environ.get("KERNEL_DEBUG") == "1":
                def dbg(name, ap):
                    d = nc.dram_tensor(
                        f"dbg_{name}", list(ap.shape), ap.dtype,
                        kind="ExternalOutput",
                    ).ap()
                    nc.sync.dma_start(out=d, in_=ap)
                dbg("qT", qT_sb)
                dbg("xkT", xkT_sb)
                dbg("xvT", xvT_sb)
                dbg("pT", pT)
                dbg("sum1", sum1)
                dbg("rbc", rbc_sb)
                dbg("attnT", attnT)
                dbg("attnT2", attnT2)
                dbg("prod", prod_sb)

    if os.environ.get("KERNEL_SKIP_LEGALIZE") != "1":
        _legalize_multiwait(nc)
    return nc


# ----------------------------------------------------------------------------
# host-side sharding / layout prep
# ----------------------------------------------------------------------------


def _np_dt(name):
    import ml_dtypes

    return {
        "bfloat16": ml_dtypes.bfloat16,
        "float8e3": ml_dtypes.float8_e3m4,
        "float16": np.float16,
        "float32": np.float32,
    }[name]


def _prep_inputs(inputs, spl, rsp):
    x = np.asarray(inputs["x"], np.float32)            # [16, 1, 4096]
    wq = np.asarray(inputs["wq"], np.float32)
    wk = np.asarray(inputs["wk"], np.float32)
    wv = np.asarray(inputs["wv"], np.float32)
    wo = np.asarray(inputs["wo"], np.float32)
    ck = np.asarray(inputs["cache_k"], np.float32)     # [16, 4096, 8, 128]
    cv = np.asarray(inputs["cache_v"], np.float32)
    shk = np.asarray(inputs["shared_cache_k"], np.float32)  # [1, 512, 8, 128]
    shv = np.asarray(inputs["shared_cache_v"], np.float32)
    cos = np.asarray(inputs["freqs_cos"], np.float32)[0]    # [64]
    sin = np.asarray(inputs["freqs_sin"], np.float32)[0]

    bdt = _np_dt("bfloat16")
    cdt = _np_dt(CACHE_DT)
    kvdt = _np_dt(WKV_DT)
    qdt = _np_dt(WQ_DT)
    odt = _np_dt(WO_DT)

    def fold_rope(w):
        # seqlen=1 decode: rope is one fixed pairwise rotation; fold it
        # into the projection columns (a host-side reparameterization)
        W = w.reshape(w.shape[0], -1, 64, 2)
        we, wo_ = W[..., 0], W[..., 1]
        return np.stack(
            [we * cos - wo_ * sin, we * sin + wo_ * cos], -1
        ).reshape(w.shape)

    wq_r = fold_rope(wq)
    wk_r = fold_rope(wk) * WS
    wv_s = wv * WS

    xm = x[:, 0, :]                                    # [16, 4096]
    xT_p = np.ascontiguousarray(
        xm.T.reshape(32, 128, B).transpose(1, 0, 2)
    ).reshape(128, 32 * B)

    BCH = rsp // 128
    in_maps = []
    for m in range(N_CORES):
        # wq': pieces of 8 k-chunks; col ((kc%8)*4+h)*128+d
        wqm = wq_r[:, 512 * m : 512 * (m + 1)]         # [4096, 512]
        wq_p = np.ascontiguousarray(
            wqm.reshape(4, 8, 128, NH * HD).transpose(0, 2, 1, 3)
        ).reshape(4, 128, 8 * NH * HD).astype(qdt)

        # wkv': col (kc*2+u)*128+d
        wkvm = np.concatenate(
            [wk_r[:, 128 * m : 128 * (m + 1)], wv_s[:, 128 * m : 128 * (m + 1)]],
            axis=1,
        )                                              # [4096, 256]
        wkv_p = np.ascontiguousarray(
            wkvm.reshape(32, 128, 256).transpose(1, 0, 2)
        ).reshape(128, 32 * 256).astype(kvdt)

        # kT: [hd, j] per batch; 4 batches per piece
        ckm = ck[:, :rsp, m, :]                        # [16, rsp, 128]
        kT_p = np.ascontiguousarray(
            ckm.transpose(0, 2, 1).reshape(4, 4, 128, rsp).transpose(0, 2, 1, 3)
        ).reshape(4, 128, 4 * rsp).astype(cdt)

        # v: [j%128, (b,c,d)]; v[b, 128c+p, d] at [p, (b*BCH+c)*128+d]
        cvm = cv[:, :rsp, m, :]                        # [16, rsp, 128]
        v_p = np.ascontiguousarray(
            cvm.reshape(4, 4, BCH, 128, 128).transpose(0, 3, 1, 2, 4)
        ).reshape(4, 128, 4 * rsp).astype(cdt)

        # wo rows for this core: [h, d, n]
        wom = wo[512 * m : 512 * (m + 1), :]           # [512, 4096]
        wo_p = np.ascontiguousarray(
            wom.reshape(NH, 128, DIM)
        ).astype(odt)

        shkT_p = shk[0, :spl, m, :].T                  # [128, spl]
        shv_p = (
            shv[0, :spl, m, :].reshape(spl // 128, 128, 128).transpose(1, 0, 2)
        ).reshape(128, spl)
        cpack = np.concatenate([xT_p, shkT_p, shv_p], axis=1).astype(bdt)

        in_maps.append(
            {
                "cpack": cpack,
                "wq": wq_p,
                "wkv": wkv_p,
                "kT": kT_p,
                "v": v_p,
                "wo": wo_p,
            }
        )
    return in_maps


# ----------------------------------------------------------------------------
# entry point
# ----------------------------------------------------------------------------

_NC_CACHE = {}


def get_nc(spl=512, rsp=1536):
    key = (spl, rsp, CACHE_DT, WKV_DT, WQ_DT, WO_DT, WO_TILES)
    if key not in _NC_CACHE:
        _patch_tile_drain()
        _install_ntff_hook()
        _NC_CACHE[key] = _build_nc(spl, rsp)
    return _NC_CACHE[key]


def prep_inputs(inputs):
    start_pos = int(inputs["start_pos"])
    spl = int(inputs["shared_prefix_length"])
    return _prep_inputs(inputs, spl, start_pos - spl)


def kernel(**inputs):
    from concourse.bass_utils import run_bass_kernel_spmd

    start_pos = int(inputs["start_pos"])
    spl = int(inputs["shared_prefix_length"])
    rsp = start_pos - spl
    nc = get_nc(spl, rsp)
    in_maps = _prep_inputs(inputs, spl, rsp)
    trace = os.environ.get("KERNEL_TRACE", "0") == "1"
    kwargs = {}
    if trace:
        kwargs = dict(
            trace=True,
            trace_cores=list(range(N_CORES)),
        )
    res = run_bass_kernel_spmd(
        nc, in_maps, core_ids=list(range(N_CORES)), **kwargs
    )
    kernel.last_result = res
    y = np.zeros((B, DIM), np.float64)
    for r in res.results:
        y += r["y"].astype(np.float64)
    return y.reshape(B, 1, DIM).astype(np.float32)
